# revision 1
# baseline (speedup 1.0000x reference)
"""Trainium2 Bass kernel for nn_Model1_52518860096440 (dense_transformer).

Reference computation (B=4, S=4096, HID=1024, H=16, DH=64):
    qkv = query @ W_qkv.T + b_qkv            # only `query` is used
    q, k, v = split(qkv); reshape to (B,S,H,DH)
    s = einsum('bshd,bsgd->bshg', q, k) / 8 + attn_mask   # per-position head mixing
    p = softmax(s, -1)
    out = einsum('bshg,bsgd->bshd', p, v).reshape(B,S,HID)

Strategy: shard the B*S = 16384 tokens across 8 cores (2048 each).
W_qkv is replicated. Per core:
  - Phase 1: QKV projection as fp32r matmuls (full PE rate at N=512),
    lhsT = query^T tiles (host-transposed), rhs = W^T tiles (host-transposed,
    attention scale 1/8 pre-folded into the q columns of W and b).
  - Phase 2: per-position 16x16 attention via fused vector ops:
    scores: per k-head g, tensor_mul (q-part x k_g broadcast) + tensor_reduce
    softmax: mask add + exp (ACT) + strided reduce + reciprocal
    AV: scalar_tensor_tensor accumulation chains, h-split across DVE/GPSIMD.
"""

from contextlib import ExitStack

import numpy as np

B, S, HID, H = 4, 4096, 1024, 16
DH = HID // H                 # 64
NCORES = 8
T = B * S                     # 16384 tokens
TC = T // NCORES              # 2048 tokens per core
P = 128                       # partitions / tokens per tile
NT = TC // P                  # 16 token tiles per core
KT = HID // P                 # 8 contraction tiles
OC = 512                      # output-chunk for QKV matmuls
NOC = 3 * HID // OC           # 6 chunks
H_DVE = 10                    # h-heads handled on DVE (rest on GPSIMD)

_compiled = {}


def _build(phase=4, sc_gps=14, av_gps=12):
    import concourse.bass as bass
    import concourse.tile as tile
    import concourse.mybir as mybir
    from concourse import bacc

    f32 = mybir.dt.float32
    f16 = mybir.dt.float16
    f32r = mybir.dt.float32r
    Alu = mybir.AluOpType
    Act = mybir.ActivationFunctionType

    nc = bacc.Bacc("TRN2", target_bir_lowering=False, debug=False,
                   num_devices=NCORES)

    xT_d = nc.dram_tensor("xT", (HID, TC), f32r, kind="ExternalInput")
    wT_d = nc.dram_tensor("wT", (HID, 3 * HID), f32r, kind="ExternalInput")
    bias_d = nc.dram_tensor("biasbc", (P, 3 * HID), f32, kind="ExternalInput")
    mask_d = nc.dram_tensor("maskp", (TC, H * H), f32, kind="ExternalInput")
    out_d = nc.dram_tensor("out", (TC, HID), f32, kind="ExternalOutput")

    with tile.TileContext(nc) as tc, ExitStack() as ctx:
        const = ctx.enter_context(tc.tile_pool(name="const", bufs=1))
        xpool = ctx.enter_context(tc.tile_pool(name="x", bufs=3))
        qkvp = ctx.enter_context(tc.tile_pool(name="qkv", bufs=3))
        work = ctx.enter_context(tc.tile_pool(name="work", bufs=4))
        opool = ctx.enter_context(tc.tile_pool(name="o", bufs=2))
        psum = ctx.enter_context(tc.tile_pool(name="ps", bufs=3, space="PSUM"))

        # ---- resident weights / bias ----
        w_tiles = []
        wT_r = wT_d[:].rearrange("(kt kp) o -> kp kt o", kp=P)
        for kt in range(KT):
            row = []
            for oc in range(NOC):
                wt = const.tile([P, OC], f32r, tag=f"w{kt}_{oc}")
                nc.sync.dma_start(wt[:], wT_r[:, kt, oc * OC:(oc + 1) * OC])
                row.append(wt)
            w_tiles.append(row)
        bias_t = const.tile([P, 3 * HID], f32)
        nc.sync.dma_start(bias_t[:], bias_d[:])
        neg4 = const.tile([P, 1], f32, tag="neg4")
        nc.vector.memset(neg4[:], -4.0)
        ones_r = const.tile([1, P], f32, tag="ones_r")
        nc.vector.memset(ones_r[:], 1.0)

        xT_r = xT_d[:].rearrange("(kt kp) t -> kp kt t", kp=P)

        for tt in range(NT):
            tsl = slice(tt * P, (tt + 1) * P)

            # ---- phase 1: QKV = x @ W^T + b ----
            x_tiles = []
            for kt in range(KT):
                xk = xpool.tile([P, P], f32r, tag=f"x{kt}")
                nc.sync.dma_start(xk[:], xT_r[:, kt, tsl])
                x_tiles.append(xk)

            qkv = qkvp.tile([P, 3 * HID], f16, tag="qkv")
            for oc in range(NOC):
                acc = psum.tile([P, OC], f32, tag="acc")
                osl = slice(oc * OC, (oc + 1) * OC)
                for kt in range(KT):
                    nc.tensor.matmul(acc[:], x_tiles[kt][:],
                                     w_tiles[kt][oc][:],
                                     start=(kt == 0), stop=False)
                # bias as a K=1 ones-row matmul accumulated into PSUM
                nc.tensor.matmul(acc[:], ones_r[:], bias_t[0:1, osl],
                                 start=False, stop=True)
                # psum -> sbuf copy on ACT (frees DVE)
                nc.scalar.copy(qkv[:, osl], acc[:])

            if phase <= 1:
                nc.sync.dma_start(out_d[tsl, :], qkv[:, 0:HID])
                continue

            qp = qkv[:, 0:HID].rearrange("p (h d) -> p h d", d=DH)

            # ---- phase 2a: scores s[t, g*16+h] = sum_d q[t,h,d] k[t,g,d] ----
            s_t = work.tile([P, H * H], f32, tag="s")
            for g in range(H):
                kg = qkv[:, HID + g * DH: HID + (g + 1) * DH]
                kg_b = kg.unsqueeze(1).broadcast_to((P, H, DH))
                tmp = work.tile([P, H, DH], f16, tag=f"tmp{g % 2}")
                mul_eng = nc.gpsimd if g < sc_gps else nc.vector
                mul_eng.tensor_mul(tmp[:], qp, kg_b)
                nc.vector.tensor_reduce(
                    s_t[:, g * H:(g + 1) * H], tmp[:],
                    axis=mybir.AxisListType.X, op=Alu.add)

            if phase <= 2:
                nc.sync.dma_start(out_d[tsl, 0:H * H], s_t[:])
                continue

            # ---- phase 2b: softmax (no max-sub; logits are O(10)) ----
            m_t = work.tile([P, H * H], f32, tag="m")
            nc.sync.dma_start(m_t[:], mask_d[tsl, :])
            sm_t = work.tile([P, H * H], f32, tag="sm")
            nc.vector.tensor_add(sm_t[:], s_t[:], m_t[:])
            e_t = work.tile([P, H * H], f16, tag="e")
            # exp(x - 4): constant shift cancels in softmax, keeps f16 finite
            nc.scalar.activation(e_t[:], sm_t[:], Act.Exp, bias=neg4[:])
            sums = work.tile([P, H], f32, tag="sums")
            nc.vector.tensor_reduce(
                sums[:], e_t[:].rearrange("p (g h) -> p h g", g=H),
                axis=mybir.AxisListType.X, op=Alu.add)
            recip = work.tile([P, H], f32, tag="recip")
            nc.vector.reciprocal(recip[:], sums[:])

            if phase <= 3:
                nc.sync.dma_start(out_d[tsl, 0:H * H], e_t[:])
                continue

            # ---- phase 2c: AV = sum_g p[t,h,g] v[t,g,:] ----
            # per h: gpsimd broadcast-mult over (g,d), DVE strided reduce over g
            vpart = qkv[:, 2 * HID:3 * HID].rearrange("p (g d) -> p g d", d=DH)
            o_t = opool.tile([P, HID], f32, tag="out")
            e3 = e_t[:].rearrange("p (g h) -> p g h", g=H)
            for h in range(H):
                # eh_b[t, g, d] = e[t, g*16+h]  (broadcast over d)
                eh_b = e3[:, :, h].unsqueeze(2).broadcast_to((P, H, DH))
                tmp = work.tile([P, H, DH], f16, tag=f"avt{h % 2}")
                mul_eng = nc.gpsimd if h < av_gps else nc.vector
                mul_eng.tensor_mul(tmp[:], vpart, eh_b)
                # reduce over g: view tmp as (p, d, g) via strides
                nc.vector.tensor_reduce(
                    o_t[:, h * DH:(h + 1) * DH],
                    tmp[:].transpose([0, 2, 1]),
                    axis=mybir.AxisListType.X, op=Alu.add)

            # ---- normalize and store ----
            r_b = recip[:].unsqueeze(2).broadcast_to((P, H, DH))
            of = opool.tile([P, HID], f32, tag="outf")
            nc.vector.tensor_mul(
                of[:].rearrange("p (h d) -> p h d", d=DH),
                o_t[:].rearrange("p (h d) -> p h d", d=DH), r_b)
            nc.sync.dma_start(out_d[tsl, :], of[:])

    nc.compile()
    return nc


def _host_prep(query, W_qkv, b_qkv, attn_mask):
    x = np.ascontiguousarray(query.reshape(T, HID), dtype=np.float32)
    xT = np.ascontiguousarray(x.T)                       # (HID, T)
    wT = np.ascontiguousarray(W_qkv.T, dtype=np.float32)  # (HID, 3*HID)
    b = np.array(b_qkv, dtype=np.float32).copy()
    scale = 1.0 / np.sqrt(DH)
    wT[:, 0:HID] *= scale
    b[0:HID] *= scale
    bias_bc = np.ascontiguousarray(np.broadcast_to(b, (P, 3 * HID)))
    # mask packed as [t, g*16+h] = attn_mask[t, h, g]
    m = np.asarray(attn_mask, dtype=np.float32).reshape(T, H, H)
    maskp = np.ascontiguousarray(m.transpose(0, 2, 1).reshape(T, H * H))
    return xT, wT, bias_bc, maskp


def kernel(query, key, value, attn_mask, W_qkv, b_qkv):
    from concourse.bass_utils import run_bass_kernel_spmd

    xT, wT, bias_bc, maskp = _host_prep(query, W_qkv, b_qkv, attn_mask)

    if "nc" not in _compiled:
        _compiled["nc"] = _build()
    nc = _compiled["nc"]

    in_maps = []
    for c in range(NCORES):
        tsl = slice(c * TC, (c + 1) * TC)
        in_maps.append({
            "xT": np.ascontiguousarray(xT[:, tsl]),
            "wT": wT,
            "biasbc": bias_bc,
            "maskp": np.ascontiguousarray(maskp[tsl, :]),
        })

    res = run_bass_kernel_spmd(nc, in_maps, core_ids=list(range(NCORES)))
    out = np.concatenate([r["out"] for r in res.results], axis=0)
    return out.reshape(B, S, HID).astype(np.float32)


if __name__ == "__main__":
    rng = np.random.default_rng(0)
    inputs = {
        "query": rng.standard_normal((B, S, HID), dtype=np.float32),
        "key": rng.standard_normal((B, S, HID), dtype=np.float32),
        "value": rng.standard_normal((B, S, HID), dtype=np.float32),
        "attn_mask": rng.standard_normal((B, S, H, H), dtype=np.float32),
        "W_qkv": (rng.standard_normal((3 * HID, HID), dtype=np.float32)
                  / np.sqrt(HID)),
        "b_qkv": rng.standard_normal((3 * HID,), dtype=np.float32) * 0.01,
    }
    out = kernel(**inputs)
    print("kernel output:", out.shape, out.dtype, np.abs(out).mean())



# revision 7
# speedup vs baseline: 1.1772x; 1.1772x over previous
"""Trainium2 Bass kernel for nn_Model1_52518860096440 (dense_transformer).

Reference computation (B=4, S=4096, HID=1024, H=16, DH=64):
    qkv = query @ W_qkv.T + b_qkv            # only `query` is used
    q, k, v = split(qkv); reshape to (B,S,H,DH)
    s = einsum('bshd,bsgd->bshg', q, k) / 8 + attn_mask   # per-position head mixing
    p = softmax(s, -1)
    out = einsum('bshg,bsgd->bshd', p, v).reshape(B,S,HID)

Strategy: shard the B*S = 16384 tokens across 8 cores (2048 each), W replicated.
Per core, per 128-token tile:
  - Phase 1: QKV projection as bf16 matmuls (full PE rate), attention scale
    1/8 folded into q columns of W; v columns host-permuted to (d,g) order so
    phase 2c reads packed-innermost. PSUM->SBUF copies on ACT (cast to f16).
  - Phase 2a: scores via one big fused f16 mul (p,h,g,d) [DVE 2x mode] +
    binary-tree reduction over d (tensor_tensor adds at 2x; level 1 on GPSIMD
    via scalar_tensor_tensor which models at 0.6 eff vs 0.42 for tensor_tensor).
  - Softmax over g: mask add (DVE), exp (ACT, bias -4), per-h sums (DVE
    reduce), reciprocal (ACT), normalize p (DVE).
  - Phase 2c: AV via big fused f16 mul (p,h,d,g) + tree over g (L1 on GPSIMD),
    final level emits f32 (h,d)-ordered output, DMA'd straight out.
Engine balance per tile: DVE ~26us, GPSIMD ~26us, PE ~11.5us, ACT ~6us.
"""

from contextlib import ExitStack

import numpy as np

B, S, HID, H = 4, 4096, 1024, 16
DH = HID // H                 # 64
NCORES = 8
T = B * S                     # 16384 tokens
TC = T // NCORES              # 2048 tokens per core
P = 128                       # partitions / tokens per tile
NT = TC // P                  # 16 token tiles per core
KT = HID // P                 # 8 contraction tiles
OC = 512                      # output-chunk for QKV matmuls
NOC = 3 * HID // OC           # 6 chunks

_compiled = {}


def _build():
    import concourse.bass as bass
    import concourse.tile as tile
    import concourse.mybir as mybir
    from concourse import bacc

    f32 = mybir.dt.float32
    f16 = mybir.dt.float16
    bf16 = mybir.dt.bfloat16
    Alu = mybir.AluOpType
    Act = mybir.ActivationFunctionType

    nc = bacc.Bacc("TRN2", target_bir_lowering=False, debug=False,
                   num_devices=NCORES)

    xT_d = nc.dram_tensor("xT", (HID, TC), bf16, kind="ExternalInput")
    wT_d = nc.dram_tensor("wT", (HID, 3 * HID), bf16, kind="ExternalInput")
    bias_d = nc.dram_tensor("biasr", (1, 3 * HID), bf16, kind="ExternalInput")
    mask_d = nc.dram_tensor("maskp", (TC, H * H), f16, kind="ExternalInput")
    out_d = nc.dram_tensor("out", (TC, HID), f32, kind="ExternalOutput")

    with tile.TileContext(nc) as tc, ExitStack() as ctx:
        const = ctx.enter_context(tc.tile_pool(name="const", bufs=1))
        xpool = ctx.enter_context(tc.tile_pool(name="x", bufs=2))
        qkvp = ctx.enter_context(tc.tile_pool(name="qkv", bufs=2))
        big = ctx.enter_context(tc.tile_pool(name="big", bufs=2))
        work = ctx.enter_context(tc.tile_pool(name="work", bufs=2))
        opool = ctx.enter_context(tc.tile_pool(name="o", bufs=2))
        psum = ctx.enter_context(tc.tile_pool(name="ps", bufs=4, space="PSUM"))

        # ---- resident weights / bias / constants ----
        w_all = const.tile([P, KT, 3 * HID], bf16)
        wT_r = wT_d[:].rearrange("(kt kp) o -> kp kt o", kp=P)
        nc.sync.dma_start(w_all[:], wT_r)
        bias_t = const.tile([1, 3 * HID], bf16)
        nc.sync.dma_start(bias_t[:], bias_d[:])
        ones_r = const.tile([1, P], bf16, tag="ones_r")
        nc.vector.memset(ones_r[:], 1.0)
        neg4 = const.tile([P, 1], f32, tag="neg4")
        nc.vector.memset(neg4[:], -4.0)
        zero_s = const.tile([P, 1], f32, tag="zero_s")
        nc.vector.memset(zero_s[:], 0.0)

        xT_r = xT_d[:].rearrange("(kt kp) t -> kp kt t", kp=P)

        for tt in range(NT):
            tsl = slice(tt * P, (tt + 1) * P)

            # ---- phase 1: QKV = x @ W^T + b (bf16 matmuls, f16 out) ----
            xk = xpool.tile([P, KT, P], bf16, tag="xk")
            nc.sync.dma_start(xk[:], xT_r[:, :, tsl])
            m_t = work.tile([P, H * H], f16, tag="m")
            nc.sync.dma_start(m_t[:], mask_d[tsl, :])

            qkv = qkvp.tile([P, 3 * HID], f16, tag="qkv")
            for oc in range(NOC):
                acc = psum.tile([P, OC], f32, tag="acc")
                osl = slice(oc * OC, (oc + 1) * OC)
                for kt in range(KT):
                    nc.tensor.matmul(acc[:], xk[:, kt, :], w_all[:, kt, osl],
                                     start=(kt == 0), stop=False)
                nc.tensor.matmul(acc[:], ones_r[:], bias_t[:, osl],
                                 start=False, stop=True)
                nc.scalar.copy(qkv[:, osl], acc[:])

            qp3 = qkv[:, 0:HID].rearrange("p (h d) -> p h d", d=DH)
            kp3 = qkv[:, HID:2 * HID].rearrange("p (g d) -> p g d", d=DH)
            vp3 = qkv[:, 2 * HID:3 * HID].rearrange("p (d g) -> p d g", g=H)

            # ---- phase 2a: s[t,h,g] = sum_d q[t,h,d] k[t,g,d] ----
            t0 = big.tile([P, H, H, DH], f16, tag="t0")
            qb = qp3.unsqueeze(2).broadcast_to((P, H, H, DH))
            kb = kp3.unsqueeze(1).broadcast_to((P, H, H, DH))
            nc.vector.tensor_tensor(t0[:], qb, kb, Alu.mult)
            # tree-reduce over d; L1 on GPSIMD (scalar_tensor_tensor: 0.6 eff)
            t1 = big.tile([P, H, H, 32], f16, tag="t1")
            nc.gpsimd.tensor_tensor(t1[:], t0[:, :, :, 0:32],
                                    t0[:, :, :, 32:64], Alu.add)
            t2 = work.tile([P, H, H, 16], f16, tag="t2")
            nc.gpsimd.tensor_tensor(t2[:], t1[:, :, :, 0:16],
                                    t1[:, :, :, 16:32], Alu.add)
            t3 = work.tile([P, H, H, 8], f16, tag="t3")
            nc.vector.tensor_tensor(t3[:], t2[:, :, :, 0:8], t2[:, :, :, 8:16],
                                    Alu.add)
            t4 = work.tile([P, H, H, 4], f16, tag="t4")
            nc.vector.tensor_tensor(t4[:], t3[:, :, :, 0:4], t3[:, :, :, 4:8],
                                    Alu.add)
            t5 = work.tile([P, H, H, 2], f16, tag="t5")
            nc.vector.tensor_tensor(t5[:], t4[:, :, :, 0:2], t4[:, :, :, 2:4],
                                    Alu.add)
            # L6 + mask add
            s0 = work.tile([P, H, H], f16, tag="t4")
            nc.vector.tensor_tensor(s0[:], t5[:, :, :, 0], t5[:, :, :, 1],
                                    Alu.add)
            sm = work.tile([P, H, H], f16, tag="t5")
            nc.vector.tensor_tensor(
                sm[:], s0[:], m_t[:].rearrange("p (h g) -> p h g", g=H),
                Alu.add)

            # ---- softmax over g ----
            e4 = work.tile([P, H, H], f16, tag="e4")
            nc.scalar.activation(e4[:], sm[:], Act.Exp, bias=neg4[:])
            sums = work.tile([P, H], f32, tag="sums")
            nc.vector.tensor_reduce(sums[:], e4[:], axis=mybir.AxisListType.X,
                                    op=Alu.add)
            recip = work.tile([P, H], f32, tag="recip")
            nc.vector.reciprocal(recip[:], sums[:])
            e4n = work.tile([P, H, H], f16, tag="m")
            rb = recip[:].unsqueeze(2).broadcast_to((P, H, H))
            nc.vector.tensor_tensor(e4n[:], e4[:], rb, Alu.mult)

            # ---- phase 2c: o[t,h,d] = sum_g p[t,h,g] v[t,g,d] ----
            u0 = big.tile([P, H, DH, H], f16, tag="t0")
            eb = e4n[:].unsqueeze(2).broadcast_to((P, H, DH, H))
            vb = vp3.unsqueeze(1).broadcast_to((P, H, DH, H))
            nc.vector.tensor_tensor(u0[:], eb, vb, Alu.mult)
            u1 = big.tile([P, H, DH, 8], f16, tag="t1")
            nc.vector.tensor_tensor(u1[:], u0[:, :, :, 0:8],
                                    u0[:, :, :, 8:16], Alu.add)
            u2 = work.tile([P, H, DH, 4], f16, tag="t2")
            HS = 8  # L2c split: h<HS on gpsimd, rest on DVE
            nc.gpsimd.tensor_tensor(u2[:, 0:HS], u1[:, 0:HS, :, 0:4],
                                    u1[:, 0:HS, :, 4:8], Alu.add)
            nc.vector.tensor_tensor(u2[:, HS:H], u1[:, HS:H, :, 0:4],
                                    u1[:, HS:H, :, 4:8], Alu.add)
            u3 = work.tile([P, H, DH, 2], f16, tag="t3")
            nc.vector.tensor_tensor(u3[:], u2[:, :, :, 0:2], u2[:, :, :, 2:4],
                                    Alu.add)
            of = opool.tile([P, H, DH], f32, tag="of")
            nc.vector.tensor_tensor(of[:], u3[:, :, :, 0], u3[:, :, :, 1],
                                    Alu.add)
            nc.sync.dma_start(out_d[tsl, :],
                              of[:].rearrange("p h d -> p (h d)"))

    nc.compile()
    return nc


def _host_prep(query, W_qkv, b_qkv, attn_mask):
    import ml_dtypes
    bf16 = ml_dtypes.bfloat16

    x = np.asarray(query, dtype=np.float32).reshape(T, HID)
    xT = np.ascontiguousarray(x.T).astype(bf16)           # (HID, T)

    W = np.asarray(W_qkv, dtype=np.float32)
    b = np.asarray(b_qkv, dtype=np.float32).copy()
    scale = 1.0 / np.sqrt(DH)
    Wq = W[0:HID] * scale                                  # (1024, 1024)
    bq = b[0:HID] * scale
    Wk = W[HID:2 * HID]
    bk = b[HID:2 * HID]
    # v rows permuted from (g,d) to (d,g) order
    Wv = W[2 * HID:3 * HID].reshape(H, DH, HID).transpose(1, 0, 2).reshape(HID, HID)
    bv = b[2 * HID:3 * HID].reshape(H, DH).T.reshape(HID)
    Wfull = np.concatenate([Wq, Wk, Wv], axis=0)           # (3072, 1024)
    wT = np.ascontiguousarray(Wfull.T).astype(bf16)        # (1024, 3072)
    biasr = np.concatenate([bq, bk, bv]).reshape(1, 3 * HID).astype(bf16)

    # mask packed as [t, h*16+g] = attn_mask[t, h, g] (natural order)
    maskp = np.asarray(attn_mask, dtype=np.float32).reshape(T, H * H)
    maskp = maskp.astype(np.float16)
    return xT, wT, biasr, maskp


def kernel(query, key, value, attn_mask, W_qkv, b_qkv):
    from concourse.bass_utils import run_bass_kernel_spmd

    xT, wT, biasr, maskp = _host_prep(query, W_qkv, b_qkv, attn_mask)

    if "nc" not in _compiled:
        _compiled["nc"] = _build()
    nc = _compiled["nc"]

    in_maps = []
    for c in range(NCORES):
        tsl = slice(c * TC, (c + 1) * TC)
        in_maps.append({
            "xT": np.ascontiguousarray(xT[:, tsl]),
            "wT": wT,
            "biasr": biasr,
            "maskp": np.ascontiguousarray(maskp[tsl, :]),
        })

    res = run_bass_kernel_spmd(nc, in_maps, core_ids=list(range(NCORES)))
    out = np.concatenate([r["out"] for r in res.results], axis=0)
    return out.reshape(B, S, HID).astype(np.float32)


if __name__ == "__main__":
    rng = np.random.default_rng(0)
    inputs = {
        "query": rng.standard_normal((B, S, HID), dtype=np.float32),
        "key": rng.standard_normal((B, S, HID), dtype=np.float32),
        "value": rng.standard_normal((B, S, HID), dtype=np.float32),
        "attn_mask": rng.standard_normal((B, S, H, H), dtype=np.float32),
        "W_qkv": (rng.standard_normal((3 * HID, HID), dtype=np.float32)
                  / np.sqrt(HID)),
        "b_qkv": rng.standard_normal((3 * HID,), dtype=np.float32) * 0.01,
    }
    out = kernel(**inputs)
    print("kernel output:", out.shape, out.dtype, np.abs(out).mean())


# revision 9
# speedup vs baseline: 1.8545x; 1.5753x over previous
"""Trainium2 Bass kernel for nn_Model1_52518860096440 (dense_transformer).

Reference computation (B=4, S=4096, HID=1024, H=16, DH=64):
    qkv = query @ W_qkv.T + b_qkv            # only `query` is used
    q, k, v = split(qkv); reshape to (B,S,H,DH)
    s = einsum('bshd,bsgd->bshg', q, k) / 8 + attn_mask   # per-position head mixing
    p = softmax(s, -1)
    out = einsum('bshg,bsgd->bshd', p, v).reshape(B,S,HID)

Strategy: shard the B*S = 16384 tokens across 8 cores (2048 each), W replicated.
Per core, per 128-token tile:
  - Phase 1: QKV projection as bf16 matmuls (full PE rate), attention scale
    1/8 folded into q columns of W; v columns host-permuted to (d,g) order so
    phase 2c reads packed-innermost. PSUM->SBUF copies on ACT (cast to f16).
  - Phase 2a: scores via one big fused f16 mul (p,h,g,d) [DVE 2x mode] +
    binary-tree reduction over d (tensor_tensor adds at 2x; level 1 on GPSIMD
    via scalar_tensor_tensor which models at 0.6 eff vs 0.42 for tensor_tensor).
  - Softmax over g: mask add (DVE), exp (ACT, bias -4), per-h sums (DVE
    reduce), reciprocal (ACT), normalize p (DVE).
  - Phase 2c: AV via big fused f16 mul (p,h,d,g) + tree over g (L1 on GPSIMD),
    final level emits f32 (h,d)-ordered output, DMA'd straight out.
Engine balance per tile: DVE ~26us, GPSIMD ~26us, PE ~11.5us, ACT ~6us.
"""

from contextlib import ExitStack

import numpy as np

B, S, HID, H = 4, 4096, 1024, 16
DH = HID // H                 # 64
NCORES = 8
T = B * S                     # 16384 tokens
TC = T // NCORES              # 2048 tokens per core
P = 128                       # partitions / tokens per tile
NT = TC // P                  # 16 token tiles per core
KT = HID // P                 # 8 contraction tiles
OC = 512                      # output-chunk for QKV matmuls
NOC = 3 * HID // OC           # 6 chunks

_compiled = {}


def _build():
    import concourse.bass as bass
    import concourse.tile as tile
    import concourse.mybir as mybir
    from concourse import bacc

    f32 = mybir.dt.float32
    f16 = mybir.dt.float16
    bf16 = mybir.dt.bfloat16
    Alu = mybir.AluOpType
    Act = mybir.ActivationFunctionType

    nc = bacc.Bacc("TRN2", target_bir_lowering=False, debug=False,
                   num_devices=NCORES)

    xT_d = nc.dram_tensor("xT", (HID, TC), bf16, kind="ExternalInput")
    wT_d = nc.dram_tensor("wT", (HID, 3 * HID), bf16, kind="ExternalInput")
    bias_d = nc.dram_tensor("biasr", (1, 3 * HID), bf16, kind="ExternalInput")
    mask_d = nc.dram_tensor("maskp", (TC, H * H), f16, kind="ExternalInput")
    out_d = nc.dram_tensor("out", (TC, HID), f32, kind="ExternalOutput")

    with tile.TileContext(nc) as tc, ExitStack() as ctx:
        const = ctx.enter_context(tc.tile_pool(name="const", bufs=1))
        xpool = ctx.enter_context(tc.tile_pool(name="x", bufs=2))
        qkvp = ctx.enter_context(tc.tile_pool(name="qkv", bufs=2))
        big = ctx.enter_context(tc.tile_pool(name="big", bufs=2))
        work = ctx.enter_context(tc.tile_pool(name="work", bufs=2))
        opool = ctx.enter_context(tc.tile_pool(name="o", bufs=1))
        psum = ctx.enter_context(tc.tile_pool(name="ps", bufs=4, space="PSUM"))

        # ---- resident weights / bias / constants ----
        w_all = const.tile([P, KT, 3 * HID], bf16)
        wT_r = wT_d[:].rearrange("(kt kp) o -> kp kt o", kp=P)
        nc.sync.dma_start(w_all[:], wT_r)
        bias_t = const.tile([1, 3 * HID], bf16)
        nc.sync.dma_start(bias_t[:], bias_d[:])
        ones_r = const.tile([1, P], bf16, tag="ones_r")
        nc.vector.memset(ones_r[:], 1.0)
        neg4 = const.tile([P, 1], f32, tag="neg4")
        nc.vector.memset(neg4[:], -4.0)

        xT_r = xT_d[:].rearrange("(kt kp) t -> kp kt t", kp=P)

        def emit_head(tt):
            """phase 1 + 2a-mul + Pool tree L1/L2 for tile tt."""
            tsl = slice(tt * P, (tt + 1) * P)
            xk = xpool.tile([P, KT, P], bf16, tag="xk")
            nc.sync.dma_start(xk[:], xT_r[:, :, tsl])
            m_t = work.tile([P, H * H], f16, tag="m")
            nc.sync.dma_start(m_t[:], mask_d[tsl, :])

            qkv = qkvp.tile([P, 3 * HID], f16, tag="qkv")
            for oc in range(NOC):
                acc = psum.tile([P, OC], f32, tag="acc")
                osl = slice(oc * OC, (oc + 1) * OC)
                for kt in range(KT):
                    nc.tensor.matmul(acc[:], xk[:, kt, :], w_all[:, kt, osl],
                                     start=(kt == 0), stop=False)
                nc.tensor.matmul(acc[:], ones_r[:], bias_t[:, osl],
                                 start=False, stop=True)
                nc.scalar.copy(qkv[:, osl], acc[:])

            qp3 = qkv[:, 0:HID].rearrange("p (h d) -> p h d", d=DH)
            kp3 = qkv[:, HID:2 * HID].rearrange("p (g d) -> p g d", d=DH)

            # 2a: big fused mul on DVE, tree L1+L2 on Pool
            t0 = big.tile([P, H, H, DH], f16, tag="t0")
            qb = qp3.unsqueeze(2).broadcast_to((P, H, H, DH))
            kb = kp3.unsqueeze(1).broadcast_to((P, H, H, DH))
            nc.vector.tensor_tensor(t0[:], qb, kb, Alu.mult)
            t1 = big.tile([P, H, H, 32], f16, tag="t1")
            nc.gpsimd.tensor_tensor(t1[:], t0[:, :, :, 0:32],
                                    t0[:, :, :, 32:64], Alu.add)
            t2 = work.tile([P, H, H, 16], f16, tag="t2")
            nc.gpsimd.tensor_tensor(t2[:], t1[:, :, :, 0:16],
                                    t1[:, :, :, 16:32], Alu.add)
            return qkv, m_t, t2

        def emit_tail(tt, state):
            """2a tree finish, softmax, 2c for tile tt."""
            qkv, m_t, t2 = state
            tsl = slice(tt * P, (tt + 1) * P)
            vp3 = qkv[:, 2 * HID:3 * HID].rearrange("p (d g) -> p d g", g=H)

            t3 = work.tile([P, H, H, 8], f16, tag="t3")
            nc.vector.tensor_tensor(t3[:], t2[:, :, :, 0:8], t2[:, :, :, 8:16],
                                    Alu.add)
            t4 = work.tile([P, H, H, 4], f16, tag="t4")
            nc.vector.tensor_tensor(t4[:], t3[:, :, :, 0:4], t3[:, :, :, 4:8],
                                    Alu.add)
            t5 = work.tile([P, H, H, 2], f16, tag="t5")
            nc.vector.tensor_tensor(t5[:], t4[:, :, :, 0:2], t4[:, :, :, 2:4],
                                    Alu.add)
            s0 = work.tile([P, H, H], f16, tag="s0")
            nc.vector.tensor_tensor(s0[:], t5[:, :, :, 0], t5[:, :, :, 1],
                                    Alu.add)
            sm = work.tile([P, H, H], f16, tag="sm")
            nc.vector.tensor_tensor(
                sm[:], s0[:], m_t[:].rearrange("p (h g) -> p h g", g=H),
                Alu.add)

            e4 = work.tile([P, H, H], f16, tag="e4")
            nc.scalar.activation(e4[:], sm[:], Act.Exp, bias=neg4[:])
            sums = work.tile([P, H], f32, tag="sums")
            nc.vector.tensor_reduce(sums[:], e4[:], axis=mybir.AxisListType.X,
                                    op=Alu.add)
            recip = work.tile([P, H], f32, tag="recip")
            nc.vector.reciprocal(recip[:], sums[:])
            e4n = work.tile([P, H, H], f16, tag="e4n")
            rb = recip[:].unsqueeze(2).broadcast_to((P, H, H))
            nc.vector.tensor_tensor(e4n[:], e4[:], rb, Alu.mult)

            u0 = big.tile([P, H, DH, H], f16, tag="t0")
            eb = e4n[:].unsqueeze(2).broadcast_to((P, H, DH, H))
            vb = vp3.unsqueeze(1).broadcast_to((P, H, DH, H))
            nc.vector.tensor_tensor(u0[:], eb, vb, Alu.mult)
            u1 = big.tile([P, H, DH, 8], f16, tag="t1")
            nc.vector.tensor_tensor(u1[:], u0[:, :, :, 0:8],
                                    u0[:, :, :, 8:16], Alu.add)
            u2 = work.tile([P, H, DH, 4], f16, tag="t2")
            HS = 8  # L2c split: h<HS on gpsimd, rest on DVE
            nc.gpsimd.tensor_tensor(u2[:, 0:HS], u1[:, 0:HS, :, 0:4],
                                    u1[:, 0:HS, :, 4:8], Alu.add)
            nc.vector.tensor_tensor(u2[:, HS:H], u1[:, HS:H, :, 0:4],
                                    u1[:, HS:H, :, 4:8], Alu.add)
            u3 = work.tile([P, H, DH, 2], f16, tag="t3")
            nc.vector.tensor_tensor(u3[:], u2[:, :, :, 0:2], u2[:, :, :, 2:4],
                                    Alu.add)
            of = opool.tile([P, H, DH], f32, tag="of")
            nc.vector.tensor_tensor(of[:], u3[:, :, :, 0], u3[:, :, :, 1],
                                    Alu.add)
            nc.sync.dma_start(out_d[tsl, :],
                              of[:].rearrange("p h d -> p (h d)"))

        prev = None
        for tt in range(NT):
            state = emit_head(tt)
            if prev is not None:
                emit_tail(tt - 1, prev)
            prev = state
        emit_tail(NT - 1, prev)

    nc.compile()
    return nc


def _host_prep(query, W_qkv, b_qkv, attn_mask):
    import ml_dtypes
    bf16 = ml_dtypes.bfloat16

    x = np.asarray(query, dtype=np.float32).reshape(T, HID)
    xT = np.ascontiguousarray(x.T).astype(bf16)           # (HID, T)

    W = np.asarray(W_qkv, dtype=np.float32)
    b = np.asarray(b_qkv, dtype=np.float32).copy()
    scale = 1.0 / np.sqrt(DH)
    Wq = W[0:HID] * scale                                  # (1024, 1024)
    bq = b[0:HID] * scale
    Wk = W[HID:2 * HID]
    bk = b[HID:2 * HID]
    # v rows permuted from (g,d) to (d,g) order
    Wv = W[2 * HID:3 * HID].reshape(H, DH, HID).transpose(1, 0, 2).reshape(HID, HID)
    bv = b[2 * HID:3 * HID].reshape(H, DH).T.reshape(HID)
    Wfull = np.concatenate([Wq, Wk, Wv], axis=0)           # (3072, 1024)
    wT = np.ascontiguousarray(Wfull.T).astype(bf16)        # (1024, 3072)
    biasr = np.concatenate([bq, bk, bv]).reshape(1, 3 * HID).astype(bf16)

    # mask packed as [t, h*16+g] = attn_mask[t, h, g] (natural order)
    maskp = np.asarray(attn_mask, dtype=np.float32).reshape(T, H * H)
    maskp = maskp.astype(np.float16)
    return xT, wT, biasr, maskp


def kernel(query, key, value, attn_mask, W_qkv, b_qkv):
    from concourse.bass_utils import run_bass_kernel_spmd

    xT, wT, biasr, maskp = _host_prep(query, W_qkv, b_qkv, attn_mask)

    if "nc" not in _compiled:
        _compiled["nc"] = _build()
    nc = _compiled["nc"]

    in_maps = []
    for c in range(NCORES):
        tsl = slice(c * TC, (c + 1) * TC)
        in_maps.append({
            "xT": np.ascontiguousarray(xT[:, tsl]),
            "wT": wT,
            "biasr": biasr,
            "maskp": np.ascontiguousarray(maskp[tsl, :]),
        })

    res = run_bass_kernel_spmd(nc, in_maps, core_ids=list(range(NCORES)))
    out = np.concatenate([r["out"] for r in res.results], axis=0)
    return out.reshape(B, S, HID).astype(np.float32)


if __name__ == "__main__":
    rng = np.random.default_rng(0)
    inputs = {
        "query": rng.standard_normal((B, S, HID), dtype=np.float32),
        "key": rng.standard_normal((B, S, HID), dtype=np.float32),
        "value": rng.standard_normal((B, S, HID), dtype=np.float32),
        "attn_mask": rng.standard_normal((B, S, H, H), dtype=np.float32),
        "W_qkv": (rng.standard_normal((3 * HID, HID), dtype=np.float32)
                  / np.sqrt(HID)),
        "b_qkv": rng.standard_normal((3 * HID,), dtype=np.float32) * 0.01,
    }
    out = kernel(**inputs)
    print("kernel output:", out.shape, out.dtype, np.abs(out).mean())


# revision 10
# speedup vs baseline: 1.8607x; 1.0033x over previous
"""Trainium2 Bass kernel for nn_Model1_52518860096440 (dense_transformer).

Reference computation (B=4, S=4096, HID=1024, H=16, DH=64):
    qkv = query @ W_qkv.T + b_qkv            # only `query` is used
    q, k, v = split(qkv); reshape to (B,S,H,DH)
    s = einsum('bshd,bsgd->bshg', q, k) / 8 + attn_mask   # per-position head mixing
    p = softmax(s, -1)
    out = einsum('bshg,bsgd->bshd', p, v).reshape(B,S,HID)

Strategy: shard the B*S = 16384 tokens across 8 cores (2048 each), W replicated.
Per core, per 128-token tile:
  - Phase 1: QKV projection as bf16 matmuls (full PE rate), attention scale
    1/8 folded into q columns of W; v columns host-permuted to (d,g) order so
    phase 2c reads packed-innermost. PSUM->SBUF copies on ACT (cast to f16).
  - Phase 2a: scores via one big fused f16 mul (p,h,g,d) [DVE 2x mode] +
    binary-tree reduction over d (tensor_tensor adds at 2x; level 1 on GPSIMD
    via scalar_tensor_tensor which models at 0.6 eff vs 0.42 for tensor_tensor).
  - Softmax over g: mask add (DVE), exp (ACT, bias -4), per-h sums (DVE
    reduce), reciprocal (ACT), normalize p (DVE).
  - Phase 2c: AV via big fused f16 mul (p,h,d,g) + tree over g (L1 on GPSIMD),
    final level emits f32 (h,d)-ordered output, DMA'd straight out.
Engine balance per tile: DVE ~26us, GPSIMD ~26us, PE ~11.5us, ACT ~6us.
"""

from contextlib import ExitStack

import numpy as np

B, S, HID, H = 4, 4096, 1024, 16
DH = HID // H                 # 64
NCORES = 8
T = B * S                     # 16384 tokens
TC = T // NCORES              # 2048 tokens per core
P = 128                       # partitions / tokens per tile
NT = TC // P                  # 16 token tiles per core
KT = HID // P                 # 8 contraction tiles
OC = 512                      # output-chunk for QKV matmuls
NOC = 3 * HID // OC           # 6 chunks

_compiled = {}


def _build():
    import concourse.bass as bass
    import concourse.tile as tile
    import concourse.mybir as mybir
    from concourse import bacc

    f32 = mybir.dt.float32
    f16 = mybir.dt.float16
    bf16 = mybir.dt.bfloat16
    Alu = mybir.AluOpType
    Act = mybir.ActivationFunctionType

    nc = bacc.Bacc("TRN2", target_bir_lowering=False, debug=False,
                   num_devices=NCORES)

    xT_d = nc.dram_tensor("xT", (HID, TC), bf16, kind="ExternalInput")
    wT_d = nc.dram_tensor("wT", (HID, 3 * HID), bf16, kind="ExternalInput")
    bias_d = nc.dram_tensor("biasr", (1, 3 * HID), bf16, kind="ExternalInput")
    mask_d = nc.dram_tensor("maskp", (TC, H * H), f16, kind="ExternalInput")
    out_d = nc.dram_tensor("out", (TC, HID), f32, kind="ExternalOutput")

    with tile.TileContext(nc) as tc, ExitStack() as ctx:
        const = ctx.enter_context(tc.tile_pool(name="const", bufs=1))
        xpool = ctx.enter_context(tc.tile_pool(name="x", bufs=2))
        qkvp = ctx.enter_context(tc.tile_pool(name="qkv", bufs=2))
        big = ctx.enter_context(tc.tile_pool(name="big", bufs=2))
        work = ctx.enter_context(tc.tile_pool(name="work", bufs=2))
        opool = ctx.enter_context(tc.tile_pool(name="o", bufs=1))
        psum = ctx.enter_context(tc.tile_pool(name="ps", bufs=4, space="PSUM"))

        # ---- resident weights / bias / constants ----
        w_all = const.tile([P, KT, 3 * HID], bf16)
        wT_r = wT_d[:].rearrange("(kt kp) o -> kp kt o", kp=P)
        nc.sync.dma_start(w_all[:], wT_r)
        bias_t = const.tile([1, 3 * HID], bf16)
        nc.sync.dma_start(bias_t[:], bias_d[:])
        ones_r = const.tile([1, P], bf16, tag="ones_r")
        nc.vector.memset(ones_r[:], 1.0)
        neg4 = const.tile([P, 1], f32, tag="neg4")
        nc.vector.memset(neg4[:], -4.0)

        xT_r = xT_d[:].rearrange("(kt kp) t -> kp kt t", kp=P)

        def emit_head(tt):
            """phase 1 + 2a-mul + Pool tree L1/L2 for tile tt."""
            tsl = slice(tt * P, (tt + 1) * P)
            xk = xpool.tile([P, KT, P], bf16, tag="xk")
            nc.sync.dma_start(xk[:], xT_r[:, :, tsl])
            m_t = work.tile([P, H * H], f16, tag="m")
            nc.sync.dma_start(m_t[:], mask_d[tsl, :])

            qkv = qkvp.tile([P, 3 * HID], f16, tag="qkv")
            for oc in range(NOC):
                acc = psum.tile([P, OC], f32, tag="acc")
                osl = slice(oc * OC, (oc + 1) * OC)
                for kt in range(KT):
                    nc.tensor.matmul(acc[:], xk[:, kt, :], w_all[:, kt, osl],
                                     start=(kt == 0), stop=False)
                nc.tensor.matmul(acc[:], ones_r[:], bias_t[:, osl],
                                 start=False, stop=True)
                nc.scalar.copy(qkv[:, osl], acc[:])

            qp3 = qkv[:, 0:HID].rearrange("p (h d) -> p h d", d=DH)
            kp3 = qkv[:, HID:2 * HID].rearrange("p (g d) -> p g d", d=DH)

            # 2a: big fused mul on DVE, tree L1+L2 on Pool
            t0 = big.tile([P, H, H, DH], f16, tag="t0")
            qb = qp3.unsqueeze(2).broadcast_to((P, H, H, DH))
            kb = kp3.unsqueeze(1).broadcast_to((P, H, H, DH))
            nc.vector.tensor_tensor(t0[:], qb, kb, Alu.mult)
            t1 = big.tile([P, H, H, 32], f16, tag="t1")
            nc.gpsimd.tensor_tensor(t1[:], t0[:, :, :, 0:32],
                                    t0[:, :, :, 32:64], Alu.add)
            t2 = work.tile([P, H, H, 16], f16, tag="t2")
            nc.gpsimd.tensor_tensor(t2[:], t1[:, :, :, 0:16],
                                    t1[:, :, :, 16:32], Alu.add)
            return qkv, m_t, t2

        def emit_tail(tt, state):
            """2a tree finish, softmax, 2c for tile tt."""
            qkv, m_t, t2 = state
            tsl = slice(tt * P, (tt + 1) * P)
            vp3 = qkv[:, 2 * HID:3 * HID].rearrange("p (d g) -> p d g", g=H)

            t3 = work.tile([P, H, H, 8], f16, tag="t3")
            nc.vector.tensor_tensor(t3[:], t2[:, :, :, 0:8], t2[:, :, :, 8:16],
                                    Alu.add)
            t4 = work.tile([P, H, H, 4], f16, tag="t4")
            nc.vector.tensor_tensor(t4[:], t3[:, :, :, 0:4], t3[:, :, :, 4:8],
                                    Alu.add)
            t5 = work.tile([P, H, H, 2], f16, tag="t5")
            nc.vector.tensor_tensor(t5[:], t4[:, :, :, 0:2], t4[:, :, :, 2:4],
                                    Alu.add)
            s0 = work.tile([P, H, H], f16, tag="s0")
            nc.vector.tensor_tensor(s0[:], t5[:, :, :, 0], t5[:, :, :, 1],
                                    Alu.add)
            sm = work.tile([P, H, H], f16, tag="sm")
            nc.vector.tensor_tensor(
                sm[:], s0[:], m_t[:].rearrange("p (h g) -> p h g", g=H),
                Alu.add)

            e4 = work.tile([P, H, H], f16, tag="e4")
            nc.scalar.activation(e4[:], sm[:], Act.Exp, bias=neg4[:])
            sums = work.tile([P, H], f32, tag="sums")
            nc.vector.tensor_reduce(sums[:], e4[:], axis=mybir.AxisListType.X,
                                    op=Alu.add)
            recip = work.tile([P, H], f32, tag="recip")
            nc.vector.reciprocal(recip[:], sums[:])
            e4n = work.tile([P, H, H], f16, tag="e4n")
            rb = recip[:].unsqueeze(2).broadcast_to((P, H, H))
            nc.vector.tensor_tensor(e4n[:], e4[:], rb, Alu.mult)

            u0 = big.tile([P, H, DH, H], f16, tag="t0")
            eb = e4n[:].unsqueeze(2).broadcast_to((P, H, DH, H))
            vb = vp3.unsqueeze(1).broadcast_to((P, H, DH, H))
            HS = 8  # L2c split: h<HS on gpsimd, rest on DVE
            nc.vector.tensor_tensor(u0[:, 0:HS], eb[:, 0:HS], vb[:, 0:HS],
                                    Alu.mult)
            u1 = big.tile([P, H, DH, 8], f16, tag="t1")
            nc.vector.tensor_tensor(u1[:, 0:HS], u0[:, 0:HS, :, 0:8],
                                    u0[:, 0:HS, :, 8:16], Alu.add)
            u2 = work.tile([P, H, DH, 4], f16, tag="t2")
            nc.gpsimd.tensor_tensor(u2[:, 0:HS], u1[:, 0:HS, :, 0:4],
                                    u1[:, 0:HS, :, 4:8], Alu.add)
            nc.vector.tensor_tensor(u0[:, HS:H], eb[:, HS:H], vb[:, HS:H],
                                    Alu.mult)
            nc.vector.tensor_tensor(u1[:, HS:H], u0[:, HS:H, :, 0:8],
                                    u0[:, HS:H, :, 8:16], Alu.add)
            nc.vector.tensor_tensor(u2[:, HS:H], u1[:, HS:H, :, 0:4],
                                    u1[:, HS:H, :, 4:8], Alu.add)
            return u2

        def emit_tail2(tt, u2):
            tsl = slice(tt * P, (tt + 1) * P)
            u3 = work.tile([P, H, DH, 2], f16, tag="t3")
            nc.vector.tensor_tensor(u3[:], u2[:, :, :, 0:2], u2[:, :, :, 2:4],
                                    Alu.add)
            of = opool.tile([P, H, DH], f32, tag="of")
            nc.vector.tensor_tensor(of[:], u3[:, :, :, 0], u3[:, :, :, 1],
                                    Alu.add)
            nc.sync.dma_start(out_d[tsl, :],
                              of[:].rearrange("p h d -> p (h d)"))

        prev = None
        pu2 = None
        for tt in range(NT):
            state = emit_head(tt)
            if pu2 is not None:
                emit_tail2(tt - 2, pu2)
            if prev is not None:
                pu2 = emit_tail(tt - 1, prev)
            prev = state
        emit_tail2(NT - 2, pu2)
        pu2 = emit_tail(NT - 1, prev)
        emit_tail2(NT - 1, pu2)

    nc.compile()
    return nc


def _host_prep(query, W_qkv, b_qkv, attn_mask):
    import ml_dtypes
    bf16 = ml_dtypes.bfloat16

    x = np.asarray(query, dtype=np.float32).reshape(T, HID)
    xT = np.ascontiguousarray(x.T).astype(bf16)           # (HID, T)

    W = np.asarray(W_qkv, dtype=np.float32)
    b = np.asarray(b_qkv, dtype=np.float32).copy()
    scale = 1.0 / np.sqrt(DH)
    Wq = W[0:HID] * scale                                  # (1024, 1024)
    bq = b[0:HID] * scale
    Wk = W[HID:2 * HID]
    bk = b[HID:2 * HID]
    # v rows permuted from (g,d) to (d,g) order
    Wv = W[2 * HID:3 * HID].reshape(H, DH, HID).transpose(1, 0, 2).reshape(HID, HID)
    bv = b[2 * HID:3 * HID].reshape(H, DH).T.reshape(HID)
    Wfull = np.concatenate([Wq, Wk, Wv], axis=0)           # (3072, 1024)
    wT = np.ascontiguousarray(Wfull.T).astype(bf16)        # (1024, 3072)
    biasr = np.concatenate([bq, bk, bv]).reshape(1, 3 * HID).astype(bf16)

    # mask packed as [t, h*16+g] = attn_mask[t, h, g] (natural order)
    maskp = np.asarray(attn_mask, dtype=np.float32).reshape(T, H * H)
    maskp = maskp.astype(np.float16)
    return xT, wT, biasr, maskp


def kernel(query, key, value, attn_mask, W_qkv, b_qkv):
    from concourse.bass_utils import run_bass_kernel_spmd

    xT, wT, biasr, maskp = _host_prep(query, W_qkv, b_qkv, attn_mask)

    if "nc" not in _compiled:
        _compiled["nc"] = _build()
    nc = _compiled["nc"]

    in_maps = []
    for c in range(NCORES):
        tsl = slice(c * TC, (c + 1) * TC)
        in_maps.append({
            "xT": np.ascontiguousarray(xT[:, tsl]),
            "wT": wT,
            "biasr": biasr,
            "maskp": np.ascontiguousarray(maskp[tsl, :]),
        })

    res = run_bass_kernel_spmd(nc, in_maps, core_ids=list(range(NCORES)))
    out = np.concatenate([r["out"] for r in res.results], axis=0)
    return out.reshape(B, S, HID).astype(np.float32)


if __name__ == "__main__":
    rng = np.random.default_rng(0)
    inputs = {
        "query": rng.standard_normal((B, S, HID), dtype=np.float32),
        "key": rng.standard_normal((B, S, HID), dtype=np.float32),
        "value": rng.standard_normal((B, S, HID), dtype=np.float32),
        "attn_mask": rng.standard_normal((B, S, H, H), dtype=np.float32),
        "W_qkv": (rng.standard_normal((3 * HID, HID), dtype=np.float32)
                  / np.sqrt(HID)),
        "b_qkv": rng.standard_normal((3 * HID,), dtype=np.float32) * 0.01,
    }
    out = kernel(**inputs)
    print("kernel output:", out.shape, out.dtype, np.abs(out).mean())


# revision 12
# speedup vs baseline: 2.1557x; 1.1585x over previous
"""Trainium2 Bass kernel for nn_Model1_52518860096440 (dense_transformer).

Reference computation (B=4, S=4096, HID=1024, H=16, DH=64):
    qkv = query @ W_qkv.T + b_qkv            # only `query` is used
    q, k, v = split(qkv); reshape to (B,S,H,DH)
    s = einsum('bshd,bsgd->bshg', q, k) / 8 + attn_mask   # per-position head mixing
    p = softmax(s, -1)
    out = einsum('bshg,bsgd->bshd', p, v).reshape(B,S,HID)

Strategy: shard the B*S = 16384 tokens across 8 cores (2048 each), W replicated.
Per core, per 128-token tile:
  - Phase 1: QKV projection as bf16 matmuls (full PE rate), attention scale
    1/8 folded into q columns of W; v columns host-permuted to (d,g) order so
    phase 2c reads packed-innermost. PSUM->SBUF copies on ACT (cast to f16).
  - Phase 2a: scores via one big fused f16 mul (p,h,g,d) [DVE 2x mode] +
    binary-tree reduction over d (tensor_tensor adds at 2x; level 1 on GPSIMD
    via scalar_tensor_tensor which models at 0.6 eff vs 0.42 for tensor_tensor).
  - Softmax over g: mask add (DVE), exp (ACT, bias -4), per-h sums (DVE
    reduce), reciprocal (ACT), normalize p (DVE).
  - Phase 2c: AV via big fused f16 mul (p,h,d,g) + tree over g (L1 on GPSIMD),
    final level emits f32 (h,d)-ordered output, DMA'd straight out.
Engine balance per tile: DVE ~26us, GPSIMD ~26us, PE ~11.5us, ACT ~6us.
"""

from contextlib import ExitStack

import numpy as np

B, S, HID, H = 4, 4096, 1024, 16
DH = HID // H                 # 64
NCORES = 8
T = B * S                     # 16384 tokens
TC = T // NCORES              # 2048 tokens per core
P = 128                       # partitions / tokens per tile
NT = TC // P                  # 16 token tiles per core
KT = HID // P                 # 8 contraction tiles
OC = 512                      # output-chunk for QKV matmuls
NOC = 3 * HID // OC           # 6 chunks

_compiled = {}


def _build():
    import concourse.bass as bass
    import concourse.tile as tile
    import concourse.mybir as mybir
    from concourse import bacc

    f32 = mybir.dt.float32
    f16 = mybir.dt.float16
    bf16 = mybir.dt.bfloat16
    Alu = mybir.AluOpType
    Act = mybir.ActivationFunctionType

    nc = bacc.Bacc("TRN2", target_bir_lowering=False, debug=False,
                   num_devices=NCORES)

    xT_d = nc.dram_tensor("xT", (HID, TC), bf16, kind="ExternalInput")
    wT_d = nc.dram_tensor("wT", (HID, 3 * HID), bf16, kind="ExternalInput")
    bias_d = nc.dram_tensor("biasr", (1, 3 * HID), bf16, kind="ExternalInput")
    mask_d = nc.dram_tensor("maskp", (TC, H * H), f16, kind="ExternalInput")
    out_d = nc.dram_tensor("out", (TC, HID), f16, kind="ExternalOutput")
    ident_d = nc.dram_tensor("ident", (P, P), f16, kind="ExternalInput")

    with tile.TileContext(nc) as tc, ExitStack() as ctx:
        const = ctx.enter_context(tc.tile_pool(name="const", bufs=1))
        xpool = ctx.enter_context(tc.tile_pool(name="x", bufs=2))
        qkvp = ctx.enter_context(tc.tile_pool(name="qkv", bufs=2))
        big = ctx.enter_context(tc.tile_pool(name="big", bufs=2))
        work = ctx.enter_context(tc.tile_pool(name="work", bufs=2))
        opool = ctx.enter_context(tc.tile_pool(name="o", bufs=1))
        psum = ctx.enter_context(tc.tile_pool(name="ps", bufs=2, space="PSUM"))
        HP = 3  # h-slices of the big muls on gpsimd

        # ---- resident weights / bias / constants ----
        w_all = const.tile([P, KT, 3 * HID], bf16)
        wT_r = wT_d[:].rearrange("(kt kp) o -> kp kt o", kp=P)
        nc.sync.dma_start(w_all[:], wT_r)
        bias_t = const.tile([1, 3 * HID], bf16)
        nc.sync.dma_start(bias_t[:], bias_d[:])
        ident = const.tile([P, P], f16, tag="ident")
        nc.sync.dma_start(ident[:], ident_d[:])
        ones_r = const.tile([1, P], bf16, tag="ones_r")
        nc.vector.memset(ones_r[:], 1.0)
        neg4 = const.tile([P, 1], f32, tag="neg4")
        nc.vector.memset(neg4[:], -4.0)

        xT_r = xT_d[:].rearrange("(kt kp) t -> kp kt t", kp=P)

        def emit_head(tt):
            """phase 1 + 2a-mul + Pool tree L1/L2 for tile tt."""
            tsl = slice(tt * P, (tt + 1) * P)
            xk = xpool.tile([P, KT, P], bf16, tag="xk")
            nc.sync.dma_start(xk[:], xT_r[:, :, tsl])
            m_t = work.tile([P, H * H], f16, tag="m")
            nc.sync.dma_start(m_t[:], mask_d[tsl, :])
            s_acc = psum.tile([P, H * H], f32, tag="s_acc")
            nc.tensor.matmul(s_acc[:], ident[:], m_t[:], start=True, stop=False)

            qkv = qkvp.tile([P, 3 * HID], f16, tag="qkv")
            for oc in range(NOC):
                acc = psum.tile([P, OC], f32, tag="acc")
                osl = slice(oc * OC, (oc + 1) * OC)
                for kt in range(KT):
                    nc.tensor.matmul(acc[:], xk[:, kt, :], w_all[:, kt, osl],
                                     start=(kt == 0), stop=False)
                nc.tensor.matmul(acc[:], ones_r[:], bias_t[:, osl],
                                 start=False, stop=True)
                nc.scalar.copy(qkv[:, osl], acc[:])

            qp3 = qkv[:, 0:HID].rearrange("p (h d) -> p h d", d=DH)
            kp3 = qkv[:, HID:2 * HID].rearrange("p (g d) -> p g d", d=DH)

            # 2a: big fused mul, h-split Pool/DVE (Pool h<HP)
            t0 = big.tile([P, H, H, DH], f16, tag="t0")
            qb = qp3.unsqueeze(2).broadcast_to((P, H, H, DH))
            kb = kp3.unsqueeze(1).broadcast_to((P, H, H, DH))
            nc.gpsimd.tensor_tensor(t0[:, 0:HP], qb[:, 0:HP], kb[:, 0:HP],
                                    Alu.mult)
            nc.vector.tensor_tensor(t0[:, HP:H], qb[:, HP:H], kb[:, HP:H],
                                    Alu.mult)
            return qkv, s_acc, t0

        def emit_tail(tt, state):
            """PE score-reduce, softmax, AV mul + PE o-reduce for tile tt."""
            qkv, s_acc, t0 = state
            tsl = slice(tt * P, (tt + 1) * P)
            vp3 = qkv[:, 2 * HID:3 * HID].rearrange("p (d g) -> p d g", g=H)

            # s[t,(h,g)] = mask + sum_d t0[t,h,g,d]  (PSUM-accumulated)
            for j in range(DH):
                nc.tensor.matmul(s_acc[:], ident[:], t0[:, :, :, j],
                                 start=False, stop=(j == DH - 1))

            e4 = work.tile([P, H, H], f16, tag="e4")
            nc.scalar.activation(e4[:], s_acc[:].rearrange("p (h g) -> p h g", g=H),
                                 Act.Exp, bias=neg4[:])
            sums = work.tile([P, H], f32, tag="sums")
            nc.vector.tensor_reduce(sums[:], e4[:], axis=mybir.AxisListType.X,
                                    op=Alu.add)
            recip = work.tile([P, H], f32, tag="recip")
            nc.vector.reciprocal(recip[:], sums[:])
            e4n = work.tile([P, H, H], f16, tag="e4n")
            rb = recip[:].unsqueeze(2).broadcast_to((P, H, H))
            nc.vector.tensor_tensor(e4n[:], e4[:], rb, Alu.mult)

            # 2c: u0[t,h,d,g] = p[t,h,g] v[t,g,d]; o = sum_g (PE-accumulated)
            u0 = big.tile([P, H, DH, H], f16, tag="t0")
            eb = e4n[:].unsqueeze(2).broadcast_to((P, H, DH, H))
            vb = vp3.unsqueeze(1).broadcast_to((P, H, DH, H))
            nc.gpsimd.tensor_tensor(u0[:, 0:HP], eb[:, 0:HP], vb[:, 0:HP],
                                    Alu.mult)
            nc.vector.tensor_tensor(u0[:, HP:H], eb[:, HP:H], vb[:, HP:H],
                                    Alu.mult)
            o_acc = psum.tile([P, HID], f32, tag="o_acc")
            u0f = u0[:].rearrange("p h d g -> p (h d) g")
            for half in range(2):
                hsl = slice(half * OC, (half + 1) * OC)
                for g in range(H):
                    nc.tensor.matmul(o_acc[:, hsl], ident[:], u0f[:, hsl, g],
                                     start=(g == 0), stop=(g == H - 1))
            of = opool.tile([P, HID], f16, tag="of")
            nc.scalar.copy(of[:, 0:OC], o_acc[:, 0:OC])
            nc.scalar.copy(of[:, OC:HID], o_acc[:, OC:HID])
            nc.sync.dma_start(out_d[tsl, :], of[:])

        prev = None
        for tt in range(NT):
            state = emit_head(tt)
            if prev is not None:
                emit_tail(tt - 1, prev)
            prev = state
        emit_tail(NT - 1, prev)

    nc.compile()
    return nc


def _host_prep(query, W_qkv, b_qkv, attn_mask):
    import ml_dtypes
    bf16 = ml_dtypes.bfloat16

    x = np.asarray(query, dtype=np.float32).reshape(T, HID)
    xT = np.ascontiguousarray(x.T).astype(bf16)           # (HID, T)

    W = np.asarray(W_qkv, dtype=np.float32)
    b = np.asarray(b_qkv, dtype=np.float32).copy()
    scale = 1.0 / np.sqrt(DH)
    Wq = W[0:HID] * scale                                  # (1024, 1024)
    bq = b[0:HID] * scale
    Wk = W[HID:2 * HID]
    bk = b[HID:2 * HID]
    # v rows permuted from (g,d) to (d,g) order
    Wv = W[2 * HID:3 * HID].reshape(H, DH, HID).transpose(1, 0, 2).reshape(HID, HID)
    bv = b[2 * HID:3 * HID].reshape(H, DH).T.reshape(HID)
    Wfull = np.concatenate([Wq, Wk, Wv], axis=0)           # (3072, 1024)
    wT = np.ascontiguousarray(Wfull.T).astype(bf16)        # (1024, 3072)
    biasr = np.concatenate([bq, bk, bv]).reshape(1, 3 * HID).astype(bf16)

    # mask packed as [t, h*16+g] = attn_mask[t, h, g] (natural order)
    maskp = np.ascontiguousarray(
        np.asarray(attn_mask, dtype=np.float32).reshape(T, H * H)).astype(np.float16)
    return xT, wT, biasr, maskp


def kernel(query, key, value, attn_mask, W_qkv, b_qkv):
    from concourse.bass_utils import run_bass_kernel_spmd

    xT, wT, biasr, maskp = _host_prep(query, W_qkv, b_qkv, attn_mask)
    ident = np.eye(P, dtype=np.float16)

    if "nc" not in _compiled:
        _compiled["nc"] = _build()
    nc = _compiled["nc"]

    in_maps = []
    for c in range(NCORES):
        tsl = slice(c * TC, (c + 1) * TC)
        in_maps.append({
            "xT": np.ascontiguousarray(xT[:, tsl]),
            "wT": wT,
            "biasr": biasr,
            "maskp": np.ascontiguousarray(maskp[tsl, :]),
            "ident": ident,
        })

    res = run_bass_kernel_spmd(nc, in_maps, core_ids=list(range(NCORES)))
    out = np.concatenate([r["out"] for r in res.results], axis=0)
    return out.reshape(B, S, HID).astype(np.float32)


if __name__ == "__main__":
    rng = np.random.default_rng(0)
    inputs = {
        "query": rng.standard_normal((B, S, HID), dtype=np.float32),
        "key": rng.standard_normal((B, S, HID), dtype=np.float32),
        "value": rng.standard_normal((B, S, HID), dtype=np.float32),
        "attn_mask": rng.standard_normal((B, S, H, H), dtype=np.float32),
        "W_qkv": (rng.standard_normal((3 * HID, HID), dtype=np.float32)
                  / np.sqrt(HID)),
        "b_qkv": rng.standard_normal((3 * HID,), dtype=np.float32) * 0.01,
    }
    out = kernel(**inputs)
    print("kernel output:", out.shape, out.dtype, np.abs(out).mean())
